# revision 2
# baseline (speedup 1.0000x reference)
"""Trainium2 Bass kernel for nn_CRHT_DGC (CTR-GCN style block), 8-core data parallel.

Per core (batch shard n=4): all BN folded on host; bf16 compute, f32 PSUM.
conv-first pipeline: xd = relu(Wd x); h = [Ws_j xd | W3 xd] (M=128 packed);
h xbar-transposed to ((t4,vp32),(n,tg,c)); graph mix = blockdiag I4(x)PA^T matmul
(K=M=128); CTRGC einsum via per-(n,c) matmuls, 4-way diagonal tile_position;
branch sums accumulate in T-mixed ACC; one xbar back-transpose; residual conv +
identity-inject + fused relu eviction; bf16 DRAM output cast to f32 on host.

Dispatch: the jax.jit(shard_map(bass_exec)) executable is built ONCE and cached;
folded params live device-resident (revalidated by memcmp against the raw param
inputs); donated output buffers are recycled device arrays, so steady-state
per-call traffic is x up (bf16) + out down (bf16) only.
"""
import numpy as np
import ml_dtypes

import concourse.bass as bass
import concourse.tile as tile
from concourse import mybir, bacc
from concourse.bass_utils import run_bass_kernel_spmd

BF16 = mybir.dt.bfloat16
F32 = mybir.dt.float32
bf = ml_dtypes.bfloat16
AF = mybir.ActivationFunctionType
OP = mybir.AluOpType

L, S, V = 3, 3, 25
CIN, COUT, INTER, REL = 64, 256, 64, 8
N, T = 32, 128
EPS = 1e-5
NCORES = 8
NL = N // NCORES          # 4
VP = 32
TG = T // 4               # 32
NTV = NL * T * V          # 12800

_CACHE = {}


def _build():
    nc = bacc.Bacc("TRN2", target_bir_lowering=False, debug=False)
    dp = nc.declare_dram_parameter
    x_ext = dp("x", [NL, CIN, T, V], BF16, isOutput=False)
    wdT_ext = dp("wdT", [L, CIN, INTER], BF16, isOutput=False)
    bd_ext = dp("bd", [L, INTER], F32, isOutput=False)
    wsT_ext = dp("wsT", [L, 2, CIN, 128], BF16, isOutput=False)
    b3c_ext = dp("b3c", [L, 128], F32, isOutput=False)
    pab_ext = dp("pab", [L, S, 128, 128], BF16, isOutput=False)
    w12T_ext = dp("w12T", [L, CIN, 40], BF16, isOutput=False)
    b12_ext = dp("b12", [L, 40], F32, isOutput=False)
    w4T_ext = dp("w4T", [L, REL, INTER], BF16, isOutput=False)
    wrT_ext = dp("wrT", [CIN, COUT], BF16, isOutput=False)
    bf_ext = dp("bfin", [2, 128], F32, isOutput=False)
    ident_ext = dp("ident", [128, 128], BF16, isOutput=False)
    out_ext = dp("out", [NL, COUT, T, V], BF16, isOutput=True)

    with tile.TileContext(nc) as tc:
        with tc.tile_pool(name="cst", bufs=1) as cst, \
             tc.tile_pool(name="big", bufs=1) as big, \
             tc.tile_pool(name="work", bufs=1) as work, \
             tc.tile_pool(name="ps", bufs=6, space="PSUM") as ps, \
             tc.tile_pool(name="ps2", bufs=2, space="PSUM") as ps2:

            x_sb = big.tile([CIN, NL, T, V], BF16, tag="x")
            nc.sync.dma_start(x_sb[:], x_ext[:].rearrange("n c t v -> c n t v"))
            wdT = cst.tile([CIN, L, INTER], BF16, tag="wdT")
            nc.sync.dma_start(wdT[:], wdT_ext[:].rearrange("l c o -> c l o"))
            wsT = cst.tile([CIN, L, 2, 128], BF16, tag="wsT")
            nc.sync.dma_start(wsT[:], wsT_ext[:].rearrange("l p c m -> c l p m"))
            pab = cst.tile([128, L, S, 128], BF16, tag="pab")
            nc.sync.dma_start(pab[:], pab_ext[:].rearrange("l s p m -> p l s m"))
            w12T = cst.tile([CIN, L, 40], BF16, tag="w12T")
            nc.sync.dma_start(w12T[:], w12T_ext[:].rearrange("l c m -> c l m"))
            w4T = cst.tile([REL, L, INTER], BF16, tag="w4T")
            nc.sync.dma_start(w4T[:], w4T_ext[:].rearrange("l r o -> r l o"))
            wrT = cst.tile([CIN, COUT], BF16, tag="wrT")
            nc.sync.dma_start(wrT[:], wrT_ext[:])
            ident = cst.tile([128, 128], BF16, tag="ident")
            nc.sync.dma_start(ident[:], ident_ext[:])
            bd_sb = cst.tile([INTER, L], F32, tag="bd")
            nc.sync.dma_start(bd_sb[:], bd_ext[:].rearrange("l o -> o l"))
            b3c_sb = cst.tile([128, L], F32, tag="b3c")
            nc.sync.dma_start(b3c_sb[:], b3c_ext[:].rearrange("l o -> o l"))
            b12_sb = cst.tile([40, L], F32, tag="b12")
            nc.sync.dma_start(b12_sb[:], b12_ext[:].rearrange("l o -> o l"))
            bf_sb = cst.tile([128, 2], F32, tag="bf")
            nc.sync.dma_start(bf_sb[:], bf_ext[:].rearrange("h o -> o h"))

            acc = big.tile([128, NL, TG, COUT], BF16, tag="acc")
            # no memset: layer-0 mix/einsum evicts overwrite every cell (incl pad rows)
            xd = big.tile([CIN, NL, T, V], BF16, tag="xd")
            h = big.tile([128, NL, T, VP], BF16, tag="h")
            nc.vector.memset(h[:, :, :, V:VP], 0.0)  # only pad cols need zeroing (NaN-safety)
            hT = big.tile([128, NL, TG, 128], BF16, tag="hT")
            h2T = hT  # shared buffer: pass1 transposes overwrite after j0/j1 mixes read
            xm = work.tile([CIN, NL, V], BF16, tag="xm")
            x1m = work.tile([REL, NL, V], F32, tag="x1m")
            x2m = work.tile([REL, NL, V], F32, tag="x2m")
            dtile = work.tile([REL, NL, V, VP], BF16, tag="d")
            nc.vector.memset(dtile[:], 0.0)
            mT4 = work.tile([128, NL, V, INTER], BF16, tag="mT4")
            red = work.tile([CIN, 64, V], BF16, tag="red")

            x_flat = x_sb[:].rearrange("c n t v -> c (n t v)")
            xd_flat = xd[:].rearrange("c n t v -> c (n t v)")
            NT400 = NTV // 400  # 32

            for i in range(L):
                # conv_down: xd = relu(Wd x + bd)
                for k in range(NTV // 512):
                    pt = ps.tile([128, 512], F32, tag="p")
                    nc.tensor.matmul(pt[0:INTER, :], wdT[:, i, :],
                                     x_flat[:, k * 512:(k + 1) * 512],
                                     start=True, stop=True)
                    dst = xd_flat[:, k * 512:(k + 1) * 512]
                    if k % 8 < 5:
                        nc.scalar.activation(dst, pt[0:INTER, :], AF.Relu,
                                             bias=bd_sb[:, i:i + 1])
                    else:
                        nc.vector.tensor_scalar(dst, pt[0:INTER, :],
                                                bd_sb[:, i:i + 1], 0.0, OP.add, OP.max)

                # xm = mean_t xd (gpsimd tree)
                for n in range(NL):
                    nc.gpsimd.tensor_add(red[:, 0:64, :], xd[:, n, 0:64, :], xd[:, n, 64:128, :])
                    nc.gpsimd.tensor_add(red[:, 0:32, :], red[:, 0:32, :], red[:, 32:64, :])
                    nc.gpsimd.tensor_add(red[:, 0:16, :], red[:, 0:16, :], red[:, 16:32, :])
                    nc.gpsimd.tensor_add(red[:, 0:8, :], red[:, 0:8, :], red[:, 8:16, :])
                    nc.gpsimd.tensor_add(red[:, 0:4, :], red[:, 0:4, :], red[:, 4:8, :])
                    nc.gpsimd.tensor_add(red[:, 0:2, :], red[:, 0:2, :], red[:, 2:4, :])
                    nc.gpsimd.tensor_add(red[:, 0, :], red[:, 0, :], red[:, 1, :])
                    nc.gpsimd.tensor_scalar(xm[:, n, :], red[:, 0, :], 1.0 / T, None, OP.mult)

                # x1 = W1 xm + b1 ; x2 = W2 xm + b2 (separate base-0 tiles)
                xmf = xm[:].rearrange("c n v -> c (n v)")
                pt1 = ps2.tile([REL, NL * V], F32, tag="q")
                nc.tensor.matmul(pt1[:], w12T[:, i, 0:REL], xmf, start=True, stop=True)
                nc.vector.tensor_scalar(x1m[:].rearrange("r n v -> r (n v)"), pt1[:],
                                        b12_sb[0:REL, i:i + 1], None, OP.add)
                pt2 = ps2.tile([REL, NL * V], F32, tag="q")
                nc.tensor.matmul(pt2[:], w12T[:, i, 32:40], xmf, start=True, stop=True)
                nc.vector.tensor_scalar(x2m[:].rearrange("r n v -> r (n v)"), pt2[:],
                                        b12_sb[32:40, i:i + 1], None, OP.add)

                # d = tanh(x1 - x2): (REL, n, u, v) into vp32-padded tile
                nc.vector.tensor_tensor(
                    dtile[:, :, :, 0:V],
                    x1m[:].rearrange("r n (u o) -> r n u o", o=1).broadcast_to([REL, NL, V, V]),
                    x2m[:].rearrange("r n (o v) -> r n o v", o=1).broadcast_to([REL, NL, V, V]),
                    OP.subtract)
                nc.scalar.activation(dtile[:, :, :, 0:V], dtile[:, :, :, 0:V], AF.Tanh)

                # mT4[vp, n, u, c] = sum_r d[r,n,u,vp] * w4T[r,c]  (then replicate x4)
                for n in range(NL):
                    for ug in range(4):
                        nu = min(8, V - ug * 8)
                        pm = ps2.tile([VP, 512], F32, tag="q")
                        for ul in range(nu):
                            u = ug * 8 + ul
                            nc.tensor.matmul(pm[:, ul * INTER:(ul + 1) * INTER],
                                             dtile[:, n, u, :], w4T[:, i, :],
                                             start=True, stop=True)
                        nc.vector.tensor_copy(
                            mT4[0:VP, n, ug * 8:ug * 8 + nu, :].rearrange("p u c -> p (u c)"),
                            pm[:, 0:nu * INTER])
                for k in range(1, 4):
                    nc.scalar.dma_start(mT4[k * 32:(k + 1) * 32, :, :, :], mT4[0:32, :, :, :])

                # h passes: p0 = [Ws0|Ws1] xd, p1 = [Ws2|W3] xd (+ [0;b3])
                def do_mix(j):
                    coff = 64 * (j % 2) if j < 2 else 0
                    for n in range(NL):
                        for kb in range(4):
                            pt = ps.tile([128, 512], F32, tag="p")
                            rhs = hT[:, n, kb * 8:(kb + 1) * 8, coff:coff + 64]
                            nc.tensor.matmul(pt[:], pab[:, i, j, :], rhs, start=True, stop=True)
                            dst = acc[:, n, kb * 8:(kb + 1) * 8, 64 * j:64 * (j + 1)]
                            ptv = pt[:].rearrange("p (t c) -> p t c", t=8)
                            if i == 0:
                                if (n * 4 + kb) % 8 < 5:
                                    nc.scalar.activation(dst, ptv, AF.Copy)
                                else:
                                    nc.vector.tensor_copy(dst, ptv)
                            else:
                                nc.vector.tensor_tensor(dst, ptv, dst, OP.add)

                for p in range(2):
                    for n in range(NL):
                        for tb in range(8):
                            k = n * 8 + tb
                            pt = ps.tile([128, 512], F32, tag="p")
                            nc.tensor.matmul(
                                pt[:, 0:400], wsT[:, i, p, :],
                                xd[:, n, tb * 16:(tb + 1) * 16, :].rearrange("c t v -> c (t v)"),
                                start=True, stop=True)
                            dst = h[:, n, tb * 16:(tb + 1) * 16, 0:V]
                            src = pt[:, 0:400].rearrange("p (t v) -> p t v", t=16)
                            if p == 1:
                                if k % 8 < 5:
                                    nc.scalar.activation(dst, src, AF.Identity,
                                                         bias=b3c_sb[:, i:i + 1])
                                else:
                                    nc.vector.tensor_scalar(dst, src, b3c_sb[:, i:i + 1],
                                                            None, OP.add)
                            else:
                                if k % 8 < 5:
                                    nc.scalar.activation(dst, src, AF.Copy)
                                else:
                                    nc.vector.tensor_copy(dst, src)
                        for tg in range(TG):
                            nc.sync.dma_start(
                                hT[:, n, tg, :],
                                h[:, n, tg * 4:(tg + 1) * 4, :].rearrange("c t v -> c (t v)"),
                                transpose=True)
                    if p == 0:
                        do_mix(0)
                        do_mix(1)
                    else:
                        do_mix(2)

                # CTRGC einsum: acc[(t4,u), (n, 192+c, tg)]
                for n in range(NL):
                    for cb in range(4):
                        pe_ = ps.tile([128, 512], F32, tag="p")
                        for cl in range(16):
                            c = cb * 16 + cl
                            for t4 in range(4):
                                nc.tensor.matmul(
                                    pe_[t4 * 32:t4 * 32 + V, cl * TG:(cl + 1) * TG],
                                    mT4[t4 * 32:t4 * 32 + V, n, :, c],
                                    h2T[t4 * 32:t4 * 32 + V, n, :, 64 + c],
                                    start=True, stop=True,
                                    tile_position=(t4 * 32, t4 * 32))
                        dst = acc[:, n, :, 192 + cb * 16:192 + (cb + 1) * 16] \
                            .rearrange("p t c -> p c t")
                        pev = pe_[:].rearrange("p (c t) -> p c t", c=16)
                        if i == 0:
                            nc.scalar.activation(dst, pev, AF.Copy)
                        else:
                            nc.vector.tensor_tensor(dst, pev, dst, OP.add)

            # final: back-transpose + residual + relu
            outc = big.tile([128, NL, TG, 4, VP], BF16, tag="hT")
            outstage = big.tile([128, NL, T, V], BF16, tag="h")
            for half in range(2):
                for n in range(NL):
                    for tg in range(TG):
                        nc.sync.dma_start(
                            outc[:, n, tg, :, :].rearrange("o a b -> o (a b)"),
                            acc[:, n, tg, half * 128:(half + 1) * 128],
                            transpose=True)
                for k in range(NT400):
                    n, tb = k // 8, k % 8
                    pt = ps.tile([128, 512], F32, tag="p")
                    nc.tensor.matmul(
                        pt[:, 0:400], wrT[:, half * 128:(half + 1) * 128],
                        x_sb[:, n, tb * 16:(tb + 1) * 16, :].rearrange("c t v -> c (t v)"),
                        start=True, stop=False)
                    nc.tensor.matmul(
                        pt[:, 0:400], ident[:],
                        outc[:, n, tb * 4:(tb + 1) * 4, :, 0:V],
                        start=False, stop=True)
                    nc.scalar.activation(
                        outstage[:, n, tb * 16:(tb + 1) * 16, :].rearrange("o t v -> o (t v)"),
                        pt[:, 0:400], AF.Relu, bias=bf_sb[:, half:half + 1])
                nc.sync.dma_start(
                    out_ext[:, half * 128:(half + 1) * 128, :, :].rearrange("n o t v -> o n t v"),
                    outstage[:])
    nc.compile()
    return nc


def _fold(inp):
    g = {k: np.asarray(v, np.float64) for k, v in inp.items()}
    cdinv = g['cdg'] / np.sqrt(g['cdv'] + EPS)
    wdT = (g['cdw'] * cdinv[:, :, None]).transpose(0, 2, 1)
    bd = (g['cdb'] - g['cdm']) * cdinv + g['cdbe']
    finv = g['bng'] / np.sqrt(g['bnv'] + EPS)
    fsh = -g['bnm'] * finv + g['bnb']
    sinv = g['sg'] / np.sqrt(g['sv'] + EPS)
    ws = g['sw'] * sinv[:, :, :, None]
    bs = (g['sb'] - g['sm']) * sinv + g['sbe']
    for j in range(S):
        ws[:, j] *= finv[64 * j:64 * (j + 1)][None, :, None]
        bs[:, j] *= finv[64 * j:64 * (j + 1)][None, :]
    assert np.abs(bs).max() < 1e-7, "nonzero subset bias unsupported"
    wsT = np.zeros((L, 2, CIN, 128))
    wsT[:, 0, :, 0:64] = ws[:, 0].transpose(0, 2, 1)
    wsT[:, 0, :, 64:128] = ws[:, 1].transpose(0, 2, 1)
    wsT[:, 1, :, 0:64] = ws[:, 2].transpose(0, 2, 1)
    wsT[:, 1, :, 64:128] = g['c3w'].transpose(0, 2, 1)
    b3c = np.zeros((L, 128))
    b3c[:, 64:128] = g['c3b']
    w4 = g['c4w'] * finv[192:256][None, :, None]
    assert np.abs(g['c4b'] * finv[192:256]).max() < 1e-7, "nonzero c4 bias unsupported"
    w12T = np.zeros((L, CIN, 40))
    w12T[:, :, 0:REL] = g['c1w'].transpose(0, 2, 1)
    w12T[:, :, 32:40] = g['c2w'].transpose(0, 2, 1)
    b12 = np.zeros((L, 40))
    b12[:, 0:REL] = g['c1b']
    b12[:, 32:40] = g['c2b']
    dinv = g['dg'] / np.sqrt(g['dv'] + EPS)
    wrT = (g['dw'] * dinv[:, None]).T
    br = (g['db'] - g['dm']) * dinv + g['dbe']
    bfin = (fsh + br).reshape(2, 128)
    pab = np.zeros((L, S, 128, 128))
    for i in range(L):
        for j in range(S):
            blk = np.zeros((VP, VP))
            blk[0:V, 0:V] = g['PA'][i, j].T
            for t4 in range(4):
                pab[i, j, t4 * 32:(t4 + 1) * 32, t4 * 32:(t4 + 1) * 32] = blk
    return {
        'wdT': np.ascontiguousarray(wdT).astype(bf), 'bd': bd.astype(np.float32),
        'wsT': wsT.astype(bf), 'b3c': b3c.astype(np.float32),
        'pab': pab.astype(bf), 'w12T': w12T.astype(bf),
        'b12': b12.astype(np.float32),
        'w4T': np.ascontiguousarray(w4.transpose(0, 2, 1)).astype(bf),
        'wrT': np.ascontiguousarray(wrT).astype(bf), 'bfin': bfin.astype(np.float32),
        'ident': np.eye(128).astype(bf),
    }


def _make_runner(nc):
    """Build the jax.jit(shard_map(bass_exec)) executable once.

    Mirrors concourse.bass2jax.run_bass_via_pjrt's multi-core path, but hoisted
    out of the per-call path so repeat calls hit the jit fast path instead of
    re-tracing + re-lowering the whole module every invocation.
    """
    import jax
    from jax.sharding import Mesh, PartitionSpec, NamedSharding
    from jax.experimental.shard_map import shard_map
    from concourse import bass2jax as b2j

    b2j.install_neuronx_cc_hook()
    assert nc.dbg_addr is None

    partition_name = nc.partition_id_tensor.name if nc.partition_id_tensor else None
    in_names, out_names, out_avals = [], [], []
    for alloc in nc.m.functions[0].allocations:
        if not isinstance(alloc, mybir.MemoryLocationSet):
            continue
        name = alloc.memorylocations[0].name
        if alloc.kind == "ExternalInput":
            if name != partition_name:
                in_names.append(name)
        elif alloc.kind == "ExternalOutput":
            out_names.append(name)
            out_avals.append(jax.core.ShapedArray(
                tuple(alloc.tensor_shape), mybir.dt.np(alloc.dtype)))
    n_params, n_outs = len(in_names), len(out_names)
    bind_in_names = in_names + out_names + ([partition_name] if partition_name else [])
    donate = tuple(range(n_params, n_params + n_outs))

    def _body(*args):
        operands = list(args)
        if partition_name is not None:
            operands.append(b2j.partition_id_tensor())
        return tuple(b2j._bass_exec_p.bind(
            *operands,
            out_avals=tuple(out_avals),
            in_names=tuple(bind_in_names),
            out_names=tuple(out_names),
            lowering_input_output_aliases=(),
            sim_require_finite=True,
            sim_require_nnan=True,
            nc=nc,
        ))

    devices = jax.devices()[:NCORES]
    mesh = Mesh(np.asarray(devices), ("core",))
    spec = PartitionSpec("core")
    sharded = jax.jit(
        shard_map(_body, mesh=mesh, in_specs=(spec,) * (n_params + n_outs),
                  out_specs=(spec,) * n_outs, check_rep=False),
        donate_argnums=donate, keep_unused=True)

    gshard = NamedSharding(mesh, spec)
    import jax.numpy as jnp
    mk_zeros = jax.jit(
        lambda: tuple(jnp.zeros((NCORES * a.shape[0], *a.shape[1:]), a.dtype)
                      for a in out_avals),
        out_shardings=(gshard,) * n_outs)
    return {
        'sharded': sharded, 'in_names': in_names, 'out_names': out_names,
        'out_avals': out_avals, 'mk_zeros': mk_zeros, 'gshard': gshard,
        'device_put': jax.device_put,
    }


def _device_params(runner, inputs):
    """Folded params as device-resident sharded arrays; refold only when the
    raw param inputs change (memcmp revalidation)."""
    raw = {k: np.asarray(v) for k, v in inputs.items() if k != 'x'}
    cached = _CACHE.get('raw_params')
    if cached is not None and all(
            np.array_equal(cached[k], raw[k]) for k in raw) and len(cached) == len(raw):
        return _CACHE['dev_params']
    params = _fold(inputs)
    dev = {k: runner['device_put'](
        np.ascontiguousarray(np.repeat(v[None], NCORES, axis=0).reshape(
            (NCORES * v.shape[0],) + v.shape[1:])), runner['gshard'])
        for k, v in params.items()}
    _CACHE['raw_params'] = raw
    _CACHE['dev_params'] = dev
    return dev


def kernel(**inputs):
    if 'nc' not in _CACHE:
        _CACHE['nc'] = _build()
        _CACHE['runner'] = _make_runner(_CACHE['nc'])
    runner = _CACHE['runner']
    dev_params = _device_params(runner, inputs)

    x = np.asarray(inputs['x'], np.float32).astype(bf)
    feed = dict(dev_params)
    feed['x'] = x
    args = [feed[name] for name in runner['in_names']]

    spare = _CACHE.pop('spare', None)
    if spare is None:
        spare = runner['mk_zeros']()
    out_arrs = runner['sharded'](*args, *spare)
    out = np.asarray(out_arrs[0])  # (N, COUT, T, V) bf16
    _CACHE['spare'] = out_arrs     # recycle as next call's donated buffers
    return out.astype(np.float32)


# revision 4
# speedup vs baseline: 1.0758x; 1.0758x over previous
"""Trainium2 Bass kernel for nn_CRHT_DGC (CTR-GCN style block), 8-core data parallel.

Device kernel (per core, batch shard n=4): all BN folded on host; bf16 compute,
f32 PSUM. conv-first pipeline: xd = relu(Wd x); h = [Ws_j xd | W3 xd] (M=128
packed); h xbar-transposed to ((t4,vp32),(n,tg,c)); graph mix = blockdiag
I4(x)PA^T matmul (K=M=128); CTRGC einsum via per-(n,c) matmuls, 4-way diagonal
tile_position; branch sums accumulate in T-mixed ACC; one xbar back-transpose;
residual conv + identity-inject + fused relu eviction; bf16 DRAM output.

Dispatch: the axon tunnel caps each TCP connection at ~75MB/s with ~80ms RTT,
but bandwidth scales linearly with connections. So 8 worker subprocesses each
hold their own axon client + a single-device jit of the bass program; each
uploads only its batch slice (0.8MB bf16), fetches only its output shard
(6.5MB bf16) over its own connection, and shift-casts bf16->f32 directly into
a /dev/shm memmap. The parent folds params (device-resident in each worker,
revalidated by memcmp), coordinates via pipes, and returns a zero-copy view of
the shared output. Single-process shard_map path kept as fallback.
"""
import os
import sys
import pickle
import shutil
import tempfile
import threading
import queue as _queue
import numpy as np
import ml_dtypes

import concourse.bass as bass
import concourse.tile as tile
from concourse import mybir, bacc

BF16 = mybir.dt.bfloat16
F32 = mybir.dt.float32
bf = ml_dtypes.bfloat16
AF = mybir.ActivationFunctionType
OP = mybir.AluOpType

L, S, V = 3, 3, 25
CIN, COUT, INTER, REL = 64, 256, 64, 8
N, T = 32, 128
EPS = 1e-5
NCORES = 8
NL = N // NCORES          # 4
VP = 32
TG = T // 4               # 32
NTV = NL * T * V          # 12800

XSHAPE = (N, CIN, T, V)
OSHAPE = (N, COUT, T, V)
OBYTES = int(np.prod(OSHAPE)) * 4

_CACHE = {}


def _build():
    nc = bacc.Bacc("TRN2", target_bir_lowering=False, debug=False)
    dp = nc.declare_dram_parameter
    x_ext = dp("x", [NL, CIN, T, V], BF16, isOutput=False)
    wdT_ext = dp("wdT", [L, CIN, INTER], BF16, isOutput=False)
    bd_ext = dp("bd", [L, INTER], F32, isOutput=False)
    wsT_ext = dp("wsT", [L, 2, CIN, 128], BF16, isOutput=False)
    b3c_ext = dp("b3c", [L, 128], F32, isOutput=False)
    pab_ext = dp("pab", [L, S, 128, 128], BF16, isOutput=False)
    w12T_ext = dp("w12T", [L, CIN, 40], BF16, isOutput=False)
    b12_ext = dp("b12", [L, 40], F32, isOutput=False)
    w4T_ext = dp("w4T", [L, REL, INTER], BF16, isOutput=False)
    wrT_ext = dp("wrT", [CIN, COUT], BF16, isOutput=False)
    bf_ext = dp("bfin", [2, 128], F32, isOutput=False)
    ident_ext = dp("ident", [128, 128], BF16, isOutput=False)
    out_ext = dp("out", [NL, COUT, T, V], BF16, isOutput=True)

    with tile.TileContext(nc) as tc:
        with tc.tile_pool(name="cst", bufs=1) as cst, \
             tc.tile_pool(name="big", bufs=1) as big, \
             tc.tile_pool(name="work", bufs=1) as work, \
             tc.tile_pool(name="ps", bufs=6, space="PSUM") as ps, \
             tc.tile_pool(name="ps2", bufs=2, space="PSUM") as ps2:

            x_sb = big.tile([CIN, NL, T, V], BF16, tag="x")
            nc.sync.dma_start(x_sb[:], x_ext[:].rearrange("n c t v -> c n t v"))
            wdT = cst.tile([CIN, L, INTER], BF16, tag="wdT")
            nc.sync.dma_start(wdT[:], wdT_ext[:].rearrange("l c o -> c l o"))
            wsT = cst.tile([CIN, L, 2, 128], BF16, tag="wsT")
            nc.sync.dma_start(wsT[:], wsT_ext[:].rearrange("l p c m -> c l p m"))
            pab = cst.tile([128, L, S, 128], BF16, tag="pab")
            nc.sync.dma_start(pab[:], pab_ext[:].rearrange("l s p m -> p l s m"))
            w12T = cst.tile([CIN, L, 40], BF16, tag="w12T")
            nc.sync.dma_start(w12T[:], w12T_ext[:].rearrange("l c m -> c l m"))
            w4T = cst.tile([REL, L, INTER], BF16, tag="w4T")
            nc.sync.dma_start(w4T[:], w4T_ext[:].rearrange("l r o -> r l o"))
            wrT = cst.tile([CIN, COUT], BF16, tag="wrT")
            nc.sync.dma_start(wrT[:], wrT_ext[:])
            ident = cst.tile([128, 128], BF16, tag="ident")
            nc.sync.dma_start(ident[:], ident_ext[:])
            bd_sb = cst.tile([INTER, L], F32, tag="bd")
            nc.sync.dma_start(bd_sb[:], bd_ext[:].rearrange("l o -> o l"))
            b3c_sb = cst.tile([128, L], F32, tag="b3c")
            nc.sync.dma_start(b3c_sb[:], b3c_ext[:].rearrange("l o -> o l"))
            b12_sb = cst.tile([40, L], F32, tag="b12")
            nc.sync.dma_start(b12_sb[:], b12_ext[:].rearrange("l o -> o l"))
            bf_sb = cst.tile([128, 2], F32, tag="bf")
            nc.sync.dma_start(bf_sb[:], bf_ext[:].rearrange("h o -> o h"))

            acc = big.tile([128, NL, TG, COUT], BF16, tag="acc")
            # no memset: layer-0 mix/einsum evicts overwrite every cell (incl pad rows)
            xd = big.tile([CIN, NL, T, V], BF16, tag="xd")
            h = big.tile([128, NL, T, VP], BF16, tag="h")
            nc.vector.memset(h[:, :, :, V:VP], 0.0)  # only pad cols need zeroing (NaN-safety)
            hT = big.tile([128, NL, TG, 128], BF16, tag="hT")
            h2T = hT  # shared buffer: pass1 transposes overwrite after j0/j1 mixes read
            xm = work.tile([CIN, NL, V], BF16, tag="xm")
            x1m = work.tile([REL, NL, V], F32, tag="x1m")
            x2m = work.tile([REL, NL, V], F32, tag="x2m")
            dtile = work.tile([REL, NL, V, VP], BF16, tag="d")
            nc.vector.memset(dtile[:], 0.0)
            mT4 = work.tile([128, NL, V, INTER], BF16, tag="mT4")
            red = work.tile([CIN, 64, V], BF16, tag="red")

            x_flat = x_sb[:].rearrange("c n t v -> c (n t v)")
            xd_flat = xd[:].rearrange("c n t v -> c (n t v)")
            NT400 = NTV // 400  # 32

            for i in range(L):
                # conv_down: xd = relu(Wd x + bd)
                for k in range(NTV // 512):
                    pt = ps.tile([128, 512], F32, tag="p")
                    nc.tensor.matmul(pt[0:INTER, :], wdT[:, i, :],
                                     x_flat[:, k * 512:(k + 1) * 512],
                                     start=True, stop=True)
                    dst = xd_flat[:, k * 512:(k + 1) * 512]
                    if k % 8 < 5:
                        nc.scalar.activation(dst, pt[0:INTER, :], AF.Relu,
                                             bias=bd_sb[:, i:i + 1])
                    else:
                        nc.vector.tensor_scalar(dst, pt[0:INTER, :],
                                                bd_sb[:, i:i + 1], 0.0, OP.add, OP.max)

                # xm = mean_t xd (gpsimd tree)
                for n in range(NL):
                    nc.gpsimd.tensor_add(red[:, 0:64, :], xd[:, n, 0:64, :], xd[:, n, 64:128, :])
                    nc.gpsimd.tensor_add(red[:, 0:32, :], red[:, 0:32, :], red[:, 32:64, :])
                    nc.gpsimd.tensor_add(red[:, 0:16, :], red[:, 0:16, :], red[:, 16:32, :])
                    nc.gpsimd.tensor_add(red[:, 0:8, :], red[:, 0:8, :], red[:, 8:16, :])
                    nc.gpsimd.tensor_add(red[:, 0:4, :], red[:, 0:4, :], red[:, 4:8, :])
                    nc.gpsimd.tensor_add(red[:, 0:2, :], red[:, 0:2, :], red[:, 2:4, :])
                    nc.gpsimd.tensor_add(red[:, 0, :], red[:, 0, :], red[:, 1, :])
                    nc.gpsimd.tensor_scalar(xm[:, n, :], red[:, 0, :], 1.0 / T, None, OP.mult)

                # x1 = W1 xm + b1 ; x2 = W2 xm + b2 (separate base-0 tiles)
                xmf = xm[:].rearrange("c n v -> c (n v)")
                pt1 = ps2.tile([REL, NL * V], F32, tag="q")
                nc.tensor.matmul(pt1[:], w12T[:, i, 0:REL], xmf, start=True, stop=True)
                nc.vector.tensor_scalar(x1m[:].rearrange("r n v -> r (n v)"), pt1[:],
                                        b12_sb[0:REL, i:i + 1], None, OP.add)
                pt2 = ps2.tile([REL, NL * V], F32, tag="q")
                nc.tensor.matmul(pt2[:], w12T[:, i, 32:40], xmf, start=True, stop=True)
                nc.vector.tensor_scalar(x2m[:].rearrange("r n v -> r (n v)"), pt2[:],
                                        b12_sb[32:40, i:i + 1], None, OP.add)

                # d = tanh(x1 - x2): (REL, n, u, v) into vp32-padded tile
                nc.vector.tensor_tensor(
                    dtile[:, :, :, 0:V],
                    x1m[:].rearrange("r n (u o) -> r n u o", o=1).broadcast_to([REL, NL, V, V]),
                    x2m[:].rearrange("r n (o v) -> r n o v", o=1).broadcast_to([REL, NL, V, V]),
                    OP.subtract)
                nc.scalar.activation(dtile[:, :, :, 0:V], dtile[:, :, :, 0:V], AF.Tanh)

                # mT4[vp, n, u, c] = sum_r d[r,n,u,vp] * w4T[r,c]  (then replicate x4)
                for n in range(NL):
                    for ug in range(4):
                        nu = min(8, V - ug * 8)
                        pm = ps2.tile([VP, 512], F32, tag="q")
                        for ul in range(nu):
                            u = ug * 8 + ul
                            nc.tensor.matmul(pm[:, ul * INTER:(ul + 1) * INTER],
                                             dtile[:, n, u, :], w4T[:, i, :],
                                             start=True, stop=True)
                        nc.vector.tensor_copy(
                            mT4[0:VP, n, ug * 8:ug * 8 + nu, :].rearrange("p u c -> p (u c)"),
                            pm[:, 0:nu * INTER])
                for k in range(1, 4):
                    nc.scalar.dma_start(mT4[k * 32:(k + 1) * 32, :, :, :], mT4[0:32, :, :, :])

                # h passes: p0 = [Ws0|Ws1] xd, p1 = [Ws2|W3] xd (+ [0;b3])
                def do_mix(j):
                    coff = 64 * (j % 2) if j < 2 else 0
                    for n in range(NL):
                        for kb in range(4):
                            pt = ps.tile([128, 512], F32, tag="p")
                            rhs = hT[:, n, kb * 8:(kb + 1) * 8, coff:coff + 64]
                            nc.tensor.matmul(pt[:], pab[:, i, j, :], rhs, start=True, stop=True)
                            dst = acc[:, n, kb * 8:(kb + 1) * 8, 64 * j:64 * (j + 1)]
                            ptv = pt[:].rearrange("p (t c) -> p t c", t=8)
                            if i == 0:
                                if (n * 4 + kb) % 8 < 5:
                                    nc.scalar.activation(dst, ptv, AF.Copy)
                                else:
                                    nc.vector.tensor_copy(dst, ptv)
                            else:
                                nc.vector.tensor_tensor(dst, ptv, dst, OP.add)

                for p in range(2):
                    for n in range(NL):
                        for tb in range(8):
                            k = n * 8 + tb
                            pt = ps.tile([128, 512], F32, tag="p")
                            nc.tensor.matmul(
                                pt[:, 0:400], wsT[:, i, p, :],
                                xd[:, n, tb * 16:(tb + 1) * 16, :].rearrange("c t v -> c (t v)"),
                                start=True, stop=True)
                            dst = h[:, n, tb * 16:(tb + 1) * 16, 0:V]
                            src = pt[:, 0:400].rearrange("p (t v) -> p t v", t=16)
                            if p == 1:
                                if k % 8 < 5:
                                    nc.scalar.activation(dst, src, AF.Identity,
                                                         bias=b3c_sb[:, i:i + 1])
                                else:
                                    nc.vector.tensor_scalar(dst, src, b3c_sb[:, i:i + 1],
                                                            None, OP.add)
                            else:
                                if k % 8 < 5:
                                    nc.scalar.activation(dst, src, AF.Copy)
                                else:
                                    nc.vector.tensor_copy(dst, src)
                        for tg in range(TG):
                            nc.sync.dma_start(
                                hT[:, n, tg, :],
                                h[:, n, tg * 4:(tg + 1) * 4, :].rearrange("c t v -> c (t v)"),
                                transpose=True)
                    if p == 0:
                        do_mix(0)
                        do_mix(1)
                    else:
                        do_mix(2)

                # CTRGC einsum: acc[(t4,u), (n, 192+c, tg)]
                for n in range(NL):
                    for cb in range(4):
                        pe_ = ps.tile([128, 512], F32, tag="p")
                        for cl in range(16):
                            c = cb * 16 + cl
                            for t4 in range(4):
                                nc.tensor.matmul(
                                    pe_[t4 * 32:t4 * 32 + V, cl * TG:(cl + 1) * TG],
                                    mT4[t4 * 32:t4 * 32 + V, n, :, c],
                                    h2T[t4 * 32:t4 * 32 + V, n, :, 64 + c],
                                    start=True, stop=True,
                                    tile_position=(t4 * 32, t4 * 32))
                        dst = acc[:, n, :, 192 + cb * 16:192 + (cb + 1) * 16] \
                            .rearrange("p t c -> p c t")
                        pev = pe_[:].rearrange("p (c t) -> p c t", c=16)
                        if i == 0:
                            nc.scalar.activation(dst, pev, AF.Copy)
                        else:
                            nc.vector.tensor_tensor(dst, pev, dst, OP.add)

            # final: back-transpose + residual + relu
            outc = big.tile([128, NL, TG, 4, VP], BF16, tag="hT")
            outstage = big.tile([128, NL, T, V], BF16, tag="h")
            for half in range(2):
                for n in range(NL):
                    for tg in range(TG):
                        nc.sync.dma_start(
                            outc[:, n, tg, :, :].rearrange("o a b -> o (a b)"),
                            acc[:, n, tg, half * 128:(half + 1) * 128],
                            transpose=True)
                for k in range(NT400):
                    n, tb = k // 8, k % 8
                    pt = ps.tile([128, 512], F32, tag="p")
                    nc.tensor.matmul(
                        pt[:, 0:400], wrT[:, half * 128:(half + 1) * 128],
                        x_sb[:, n, tb * 16:(tb + 1) * 16, :].rearrange("c t v -> c (t v)"),
                        start=True, stop=False)
                    nc.tensor.matmul(
                        pt[:, 0:400], ident[:],
                        outc[:, n, tb * 4:(tb + 1) * 4, :, 0:V],
                        start=False, stop=True)
                    nc.scalar.activation(
                        outstage[:, n, tb * 16:(tb + 1) * 16, :].rearrange("o t v -> o (t v)"),
                        pt[:, 0:400], AF.Relu, bias=bf_sb[:, half:half + 1])
                nc.sync.dma_start(
                    out_ext[:, half * 128:(half + 1) * 128, :, :].rearrange("n o t v -> o n t v"),
                    outstage[:])
    nc.compile()
    return nc


def _fold(inp):
    g = {k: np.asarray(v, np.float64) for k, v in inp.items()}
    cdinv = g['cdg'] / np.sqrt(g['cdv'] + EPS)
    wdT = (g['cdw'] * cdinv[:, :, None]).transpose(0, 2, 1)
    bd = (g['cdb'] - g['cdm']) * cdinv + g['cdbe']
    finv = g['bng'] / np.sqrt(g['bnv'] + EPS)
    fsh = -g['bnm'] * finv + g['bnb']
    sinv = g['sg'] / np.sqrt(g['sv'] + EPS)
    ws = g['sw'] * sinv[:, :, :, None]
    bs = (g['sb'] - g['sm']) * sinv + g['sbe']
    for j in range(S):
        ws[:, j] *= finv[64 * j:64 * (j + 1)][None, :, None]
        bs[:, j] *= finv[64 * j:64 * (j + 1)][None, :]
    assert np.abs(bs).max() < 1e-7, "nonzero subset bias unsupported"
    wsT = np.zeros((L, 2, CIN, 128))
    wsT[:, 0, :, 0:64] = ws[:, 0].transpose(0, 2, 1)
    wsT[:, 0, :, 64:128] = ws[:, 1].transpose(0, 2, 1)
    wsT[:, 1, :, 0:64] = ws[:, 2].transpose(0, 2, 1)
    wsT[:, 1, :, 64:128] = g['c3w'].transpose(0, 2, 1)
    b3c = np.zeros((L, 128))
    b3c[:, 64:128] = g['c3b']
    w4 = g['c4w'] * finv[192:256][None, :, None]
    assert np.abs(g['c4b'] * finv[192:256]).max() < 1e-7, "nonzero c4 bias unsupported"
    w12T = np.zeros((L, CIN, 40))
    w12T[:, :, 0:REL] = g['c1w'].transpose(0, 2, 1)
    w12T[:, :, 32:40] = g['c2w'].transpose(0, 2, 1)
    b12 = np.zeros((L, 40))
    b12[:, 0:REL] = g['c1b']
    b12[:, 32:40] = g['c2b']
    dinv = g['dg'] / np.sqrt(g['dv'] + EPS)
    wrT = (g['dw'] * dinv[:, None]).T
    br = (g['db'] - g['dm']) * dinv + g['dbe']
    bfin = (fsh + br).reshape(2, 128)
    pab = np.zeros((L, S, 128, 128))
    for i in range(L):
        for j in range(S):
            blk = np.zeros((VP, VP))
            blk[0:V, 0:V] = g['PA'][i, j].T
            for t4 in range(4):
                pab[i, j, t4 * 32:(t4 + 1) * 32, t4 * 32:(t4 + 1) * 32] = blk
    return {
        'wdT': np.ascontiguousarray(wdT).astype(bf), 'bd': bd.astype(np.float32),
        'wsT': wsT.astype(bf), 'b3c': b3c.astype(np.float32),
        'pab': pab.astype(bf), 'w12T': w12T.astype(bf),
        'b12': b12.astype(np.float32),
        'w4T': np.ascontiguousarray(w4.transpose(0, 2, 1)).astype(bf),
        'wrT': np.ascontiguousarray(wrT).astype(bf), 'bfin': bfin.astype(np.float32),
        'ident': np.eye(128).astype(bf),
    }


def _scan_io(nc):
    """ExternalInput/Output names + avals from the finalized module."""
    partition_name = nc.partition_id_tensor.name if nc.partition_id_tensor else None
    in_names, out_names, out_shapes = [], [], []
    for alloc in nc.m.functions[0].allocations:
        if not isinstance(alloc, mybir.MemoryLocationSet):
            continue
        name = alloc.memorylocations[0].name
        if alloc.kind == "ExternalInput":
            if name != partition_name:
                in_names.append(name)
        elif alloc.kind == "ExternalOutput":
            out_names.append(name)
            out_shapes.append((tuple(alloc.tensor_shape), mybir.dt.np(alloc.dtype)))
    return partition_name, in_names, out_names, out_shapes


def _bind_body(nc, partition_name, in_names, out_names, out_shapes):
    import jax
    from concourse import bass2jax as b2j
    out_avals = [jax.core.ShapedArray(s, d) for s, d in out_shapes]
    bind_in_names = in_names + out_names + ([partition_name] if partition_name else [])

    def _body(*args):
        operands = list(args)
        if partition_name is not None:
            operands.append(b2j.partition_id_tensor())
        return tuple(b2j._bass_exec_p.bind(
            *operands,
            out_avals=tuple(out_avals),
            in_names=tuple(bind_in_names),
            out_names=tuple(out_names),
            lowering_input_output_aliases=(),
            sim_require_finite=True,
            sim_require_nnan=True,
            nc=nc,
        ))
    return _body


# ---------------------------------------------------------------- worker

def worker_main(w, tmpdir):
    import jax
    from concourse import bass2jax as b2j
    b2j.install_neuronx_cc_hook()

    nc = _build()
    partition_name, in_names, out_names, out_shapes = _scan_io(nc)
    body = _bind_body(nc, partition_name, in_names, out_names, out_shapes)
    n_params, n_outs = len(in_names), len(out_names)
    donate = tuple(range(n_params, n_params + n_outs))
    jf = jax.jit(body, donate_argnums=donate, keep_unused=True)
    dev = jax.devices()[w]
    xmm = np.memmap(os.path.join(tmpdir, "x.bin"), np.float32, 'r', shape=XSHAPE)
    spare = tuple(jax.device_put(np.zeros(s, d), dev) for s, d in out_shapes)
    dev_params = None
    lo, hi = w * NL, (w + 1) * NL

    def run_once():
        nonlocal spare
        xs = jax.device_put(np.ascontiguousarray(xmm[lo:hi]).astype(bf), dev)
        feed = dict(dev_params)
        feed['x'] = xs
        args = [feed[nm] for nm in in_names]
        outs = jf(*args, *spare)
        sh = outs[0]
        try:
            sh.copy_to_host_async()
        except Exception:
            pass
        h = np.asarray(sh)
        spare = outs
        return h

    print("ready", flush=True)
    for line in sys.stdin:
        cmd = line.split()
        if not cmd or cmd[0] == "exit":
            break
        try:
            if cmd[0] == "init":
                with open(os.path.join(tmpdir, "params.pkl"), "rb") as f:
                    params = pickle.load(f)
                dev_params = {k: jax.device_put(v, dev) for k, v in params.items()}
                jax.block_until_ready(list(dev_params.values()))
                run_once()  # warm trace/compile + one full roundtrip
                print("done -1", flush=True)
            elif cmd[0] == "run":
                seq = int(cmd[1])
                h = run_once()
                omm = np.memmap(os.path.join(tmpdir, f"out_{seq}.bin"),
                                np.float32, 'r+', shape=OSHAPE)
                dst = omm[lo:hi]
                np.left_shift(h.view(np.uint16).astype(np.uint32), 16,
                              out=dst.view(np.uint32))
                del dst, omm
                print(f"done {seq}", flush=True)
            else:
                print(f"err unknown command {cmd[0]}", flush=True)
        except Exception as e:
            import traceback
            traceback.print_exc(file=sys.stderr)
            sys.stderr.flush()
            print("err " + repr(e).replace("\n", " "), flush=True)


# ------------------------------------------------------------- main side

def _reader_thread(proc, q):
    for line in proc.stdout:
        q.put(line.strip())
    q.put(None)


def _wait_msg(st, idx, want, timeout):
    while True:
        msg = st['queues'][idx].get(timeout=timeout)
        if msg is None:
            raise RuntimeError(f"worker {idx} died")
        if msg.startswith("err"):
            raise RuntimeError(f"worker {idx}: {msg}")
        if msg == want:
            return
        # stale message (e.g. from an aborted call) — keep draining


def _teardown_mp():
    st = _CACHE.pop('mp', None)
    if st is None:
        return
    for p in st['procs']:
        try:
            p.stdin.write("exit\n")
            p.stdin.flush()
        except Exception:
            pass
    for p in st['procs']:
        try:
            p.terminate()
        except Exception:
            pass
    shutil.rmtree(st['dir'], ignore_errors=True)


def _setup_mp():
    import subprocess, atexit
    base = "/dev/shm" if os.path.isdir("/dev/shm") else tempfile.gettempdir()
    tmpdir = tempfile.mkdtemp(prefix="crht_", dir=base)
    xmm = np.memmap(os.path.join(tmpdir, "x.bin"), np.float32, 'w+', shape=XSHAPE)
    procs, queues = [], []
    script = os.path.abspath(__file__)
    for w in range(NCORES):
        errf = open(os.path.join(tmpdir, f"worker_{w}.err"), "w")
        p = subprocess.Popen(
            [sys.executable, "-u", script, "--worker", str(w), tmpdir],
            stdin=subprocess.PIPE, stdout=subprocess.PIPE, stderr=errf,
            text=True, bufsize=1)
        procs.append(p)
        q = _queue.Queue()
        threading.Thread(target=_reader_thread, args=(p, q), daemon=True).start()
        queues.append(q)
    st = {'dir': tmpdir, 'procs': procs, 'queues': queues, 'xmm': xmm,
          'seq': 0, 'raw_params': None}
    _CACHE['mp'] = st
    atexit.register(_teardown_mp)
    for w in range(NCORES):
        _wait_msg(st, w, "ready", timeout=1800)
    return st


def _send_all(st, msg):
    for p in st['procs']:
        p.stdin.write(msg + "\n")
        p.stdin.flush()


def _mp_init_params(st, inputs):
    raw = {k: np.asarray(v) for k, v in inputs.items() if k != 'x'}
    old = st['raw_params']
    if old is not None and len(old) == len(raw) and \
            all(k in old and np.array_equal(old[k], raw[k]) for k in raw):
        return
    params = _fold(inputs)
    with open(os.path.join(st['dir'], "params.pkl"), "wb") as f:
        pickle.dump(params, f, protocol=4)
    # worker 0 warms first so a cold NEFF compile happens once, not 8x in parallel
    st['procs'][0].stdin.write("init\n")
    st['procs'][0].stdin.flush()
    _wait_msg(st, 0, "done -1", timeout=3600)
    for w in range(1, NCORES):
        st['procs'][w].stdin.write("init\n")
        st['procs'][w].stdin.flush()
    for w in range(1, NCORES):
        _wait_msg(st, w, "done -1", timeout=3600)
    st['raw_params'] = raw


def _kernel_mp(inputs):
    st = _CACHE.get('mp')
    if st is None:
        st = _setup_mp()
    x = np.asarray(inputs['x'], np.float32)
    np.copyto(st['xmm'], x)
    _mp_init_params(st, inputs)
    st['seq'] += 1
    seq = st['seq']
    path = os.path.join(st['dir'], f"out_{seq}.bin")
    with open(path, "wb") as f:
        f.truncate(OBYTES)
    _send_all(st, f"run {seq}")
    for w in range(NCORES):
        _wait_msg(st, w, f"done {seq}", timeout=300)
    omm = np.memmap(path, np.float32, 'r+', shape=OSHAPE)
    os.unlink(path)
    return np.asarray(omm)


# -------------------------------------------- single-process fallback path

def _make_runner_sp():
    import jax
    from jax.sharding import Mesh, PartitionSpec, NamedSharding
    from jax.experimental.shard_map import shard_map
    from concourse import bass2jax as b2j

    b2j.install_neuronx_cc_hook()
    nc = _build()
    partition_name, in_names, out_names, out_shapes = _scan_io(nc)
    body = _bind_body(nc, partition_name, in_names, out_names, out_shapes)
    n_params, n_outs = len(in_names), len(out_names)
    donate = tuple(range(n_params, n_params + n_outs))

    devices = jax.devices()[:NCORES]
    mesh = Mesh(np.asarray(devices), ("core",))
    spec = PartitionSpec("core")
    sharded = jax.jit(
        shard_map(body, mesh=mesh, in_specs=(spec,) * (n_params + n_outs),
                  out_specs=(spec,) * n_outs, check_rep=False),
        donate_argnums=donate, keep_unused=True)
    gshard = NamedSharding(mesh, spec)
    import jax.numpy as jnp
    mk_zeros = jax.jit(
        lambda: tuple(jnp.zeros((NCORES * s[0], *s[1:]), d) for s, d in out_shapes),
        out_shardings=(gshard,) * n_outs)
    return {'nc': nc, 'sharded': sharded, 'in_names': in_names,
            'mk_zeros': mk_zeros, 'gshard': gshard, 'device_put': jax.device_put}


def _kernel_sp(inputs):
    runner = _CACHE.get('sp_runner')
    if runner is None:
        runner = _CACHE['sp_runner'] = _make_runner_sp()
    raw = {k: np.asarray(v) for k, v in inputs.items() if k != 'x'}
    cached = _CACHE.get('sp_raw')
    if cached is None or len(cached) != len(raw) or not all(
            k in cached and np.array_equal(cached[k], raw[k]) for k in raw):
        params = _fold(inputs)
        _CACHE['sp_params'] = {
            k: runner['device_put'](np.ascontiguousarray(
                np.repeat(v[None], NCORES, axis=0).reshape(
                    (NCORES * v.shape[0],) + v.shape[1:])), runner['gshard'])
            for k, v in params.items()}
        _CACHE['sp_raw'] = raw
    feed = dict(_CACHE['sp_params'])
    feed['x'] = np.asarray(inputs['x'], np.float32).astype(bf)
    args = [feed[name] for name in runner['in_names']]
    spare = _CACHE.pop('sp_spare', None)
    if spare is None:
        spare = runner['mk_zeros']()
    out_arrs = runner['sharded'](*args, *spare)
    out = np.asarray(out_arrs[0])
    _CACHE['sp_spare'] = out_arrs
    res = np.empty(OSHAPE, np.float32)
    np.left_shift(out.view(np.uint16).astype(np.uint32), 16,
                  out=res.view(np.uint32))
    return res


def kernel(**inputs):
    if not os.environ.get("CRHT_FORCE_SP"):
        try:
            return _kernel_mp(inputs)
        except Exception:
            import traceback
            traceback.print_exc(file=sys.stderr)
            _teardown_mp()
    return _kernel_sp(inputs)


if __name__ == "__main__":
    if len(sys.argv) >= 4 and sys.argv[1] == "--worker":
        worker_main(int(sys.argv[2]), sys.argv[3])
    else:
        sys.exit("usage: kernel.py --worker <idx> <tmpdir>")
